# revision 3
# baseline (speedup 1.0000x reference)
"""Causal linear attention (B=2, H=8, T=2048, D=64) on 8 Trainium2 NeuronCores.

Sharding: 16 (batch, head) pairs split 2-per-core; per (b,h) a chunked scan
over T in chunks of C=128:
  out_chunk = tril(Qp Kp^T) @ [V|1]  +  Qp @ S ,   S += Kp^T @ [V|1]
with Qp/Kp = elu(.)+1 feature maps; the ones-column of V produces the
normalizer z in the last output column.  The kernel emits (num|z) in bf16;
the final out = num/z division happens on the host during unpacking.

Structure vs the original version:
 - q arrives host-transposed (d on partitions) -> no on-chip q transpose.
 - Kp^T comes from SBUF->SBUF DMA-transpose of feature-mapped k (no PE
   transposes at all).
 - Running state S lives in TWO psum banks (even/odd chunks) so the serial
   state->evac chain only recurs every other chunk; the cross-head garbage
   of the single full-width state matmul is simply never read.
 - Chunks are processed in pairs sharing a 4-bank psum tile, so the A^T
   mask-evac and the output evac each run once per pair (half the op
   overhead); evacs alternate between Scalar(ACT) and Vector engines.
"""

import sys

sys.path.insert(0, "/opt/trn_rl_repo")

from contextlib import ExitStack

import numpy as np
import ml_dtypes

import concourse.bass as bass
import concourse.bacc as bacc
import concourse.mybir as mybir
import concourse.tile as tile
from concourse.bass_utils import run_bass_kernel_spmd

B, H, T, D = 2, 8, 2048, 64
N_CORES = 8
PAIRS = B * H                  # 16 (batch, head) pairs
PPC = PAIRS // N_CORES         # 2 pairs per core
C = 128                        # chunk (= partition) size
NCH = T // C                   # 16 chunks
DV = D + 1                     # value dim incl normalizer ones-column
BANK_F32 = 512                 # fp32 slots per 2 KiB PSUM bank
QW = C                         # qT cols per chunk (128)
KW = PPC * D                   # k cols per chunk (128)
VW = PPC * DV                  # v cols per chunk (130)
CW = QW + KW + VW              # total dram cols per chunk (386)

F32 = mybir.dt.float32
BF16 = mybir.dt.bfloat16
AF = mybir.ActivationFunctionType
ALU = mybir.AluOpType

BF = ml_dtypes.bfloat16

# DMA load segments (chunk ranges) and feature-map segments.
LOAD_SEGS = [(0, 2), (2, 4), (4, 6), (6, 10), (10, 16)]
FM_SEGS = [(0, 2), (2, 4), (4, 6), (6, 10), (10, 16)]
MAXSEG = max(s1 - s0 for s0, s1 in FM_SEGS)
# emission schedule: chunk-pair index -> list of (kind, seg)
PRE_SEGS = 2   # load+fm segments emitted before the pair loop
STAGGER = {1: [2], 2: [3], 3: [4]}
# output store batches: pair index after which each fires -> chunk range
OUT_BATCHES = {3: (0, 8), 5: (8, 12), 6: (12, 14), 7: (14, 16)}

_CACHE = {}


def _build():
    nc = bacc.Bacc(None, target_bir_lowering=False)
    qkv_d = nc.dram_tensor("qkv", [C, NCH, CW], BF16, kind="ExternalInput")
    o_d = nc.dram_tensor("out", [C, NCH, PPC, DV], BF16, kind="ExternalOutput")

    with ExitStack() as ctx:
        tc = ctx.enter_context(tile.TileContext(nc))
        consts = ctx.enter_context(tc.tile_pool(name="consts", bufs=1))
        loads = ctx.enter_context(tc.tile_pool(name="loads", bufs=1))
        fmp = ctx.enter_context(tc.tile_pool(name="fmp", bufs=1))
        spoolE = ctx.enter_context(tc.tile_pool(name="spoolE", bufs=2))
        spoolO = ctx.enter_context(tc.tile_pool(name="spoolO", bufs=2))
        ampool = ctx.enter_context(tc.tile_pool(name="ampool", bufs=2))
        ps_s = ctx.enter_context(tc.tile_pool(name="ps_s", bufs=1, space="PSUM"))
        ps_o = ctx.enter_context(tc.tile_pool(name="ps_o", bufs=1, space="PSUM"))

        # A^T[s,t] keeps s<=t: triu mask replicated per head (bf16).
        mask_d = nc.inline_tensor(
            np.ascontiguousarray(
                np.broadcast_to(
                    np.triu(np.ones((C, C), np.float32))[:, None, :], (C, PPC, C)
                )
            ).astype(BF),
            name="mask_c",
        )
        mask = consts.tile([C, PPC, C], BF16, tag="mask")

        qkvf = loads.tile([C, NCH, CW], BF16, tag="qkvf", name="qkvf")
        qTp = loads.tile([C, NCH, QW], BF16, tag="qTp", name="qTp")
        kp = loads.tile([C, NCH, KW], BF16, tag="kp", name="kp")
        kTp = loads.tile([C, NCH, C], BF16, tag="kTp", name="kTp")
        outf = loads.tile([C, NCH, PPC, DV], BF16, tag="outf", name="outf")

        def emit_load(s0, s1, eng=None):
            (eng or nc.sync).dma_start(out=qkvf[:, s0:s1], in_=qkv_d[:, s0:s1])

        def emit_fm(s0, s1):
            nseg = s1 - s0
            qkw = QW + KW
            src_ap = bass.AP(
                tensor=qkvf.tensor,
                offset=qkvf.offset + s0 * CW,
                ap=[qkvf.ap[0], [CW, nseg], [1, qkw]],
            )
            # feature map: elu(x)+1 == max(min(exp(x), 1), x+1)
            e = fmp.tile([C, MAXSEG, qkw], BF16, tag="e", name="e", bufs=2)
            nc.scalar.activation(out=e[:, :nseg, :], in_=src_ap, func=AF.Exp)
            a = fmp.tile([C, MAXSEG, qkw], BF16, tag="a", name="a", bufs=2)
            nc.vector.tensor_scalar_add(out=a[:, :nseg, :], in0=src_ap, scalar1=1.0)
            nc.vector.scalar_tensor_tensor(
                out=qTp[:, s0:s1, :],
                in0=e[:, :nseg, 0:QW],
                scalar=1.0,
                in1=a[:, :nseg, 0:QW],
                op0=ALU.min,
                op1=ALU.max,
            )
            nc.vector.scalar_tensor_tensor(
                out=kp[:, s0:s1, :],
                in0=e[:, :nseg, QW : QW + KW],
                scalar=1.0,
                in1=a[:, :nseg, QW : QW + KW],
                op0=ALU.min,
                op1=ALU.max,
            )
            # Kp^T via SBUF->SBUF DMA transpose, one per chunk.
            for n in range(s0, s1):
                nc.sync.dma_start_transpose(out=kTp[:, n, :], in_=kp[:, n, :])

        for i in range(PRE_SEGS):
            emit_load(*LOAD_SEGS[i])
            if i == 0:
                nc.scalar.dma_start(out=mask, in_=mask_d[:, :])
            emit_fm(*FM_SEGS[i])

        # Running state: 2 psum banks (even/odd chunks); head h occupies
        # partitions [64h,64h+64) x cols [65h,65h+65) (cross blocks garbage).
        s_ps = ps_s.tile([C, 2, BANK_F32], F32, tag="s", name="s_ps")
        # Pair-shared output/AT psum: [slot, head] -> one bank each.
        opat = ps_o.tile([C, 2, PPC, BANK_F32], F32, tag="opat", name="opat")
        s_sb = {}  # n -> sbuf snapshot of parity-prefix through chunk n
        vbase = QW + KW

        def vap(n, h):
            return qkvf[:, n, vbase + h * DV : vbase + (h + 1) * DV]

        for m in range(NCH // 2):
            n0, n1 = 2 * m, 2 * m + 1
            for seg in STAGGER.get(m, []):
                emit_load(*LOAD_SEGS[seg])
                emit_fm(*FM_SEGS[seg])

            # State updates + snapshots (evac alternates ACT/DVE).
            for n in (n0, n1):
                if n >= NCH - 1:
                    continue
                par = n % 2
                nc.tensor.matmul(
                    s_ps[:, par, 0:VW],
                    kp[:, n, :],
                    qkvf[:, n, vbase:CW],
                    start=(n < 2),
                    stop=True,
                    skip_group_check=True,
                )
                snew = (spoolE if par == 0 else spoolO).tile(
                    [C, VW], BF16, tag=f"sb{par}", name=f"sb{par}"
                )
                if par == 0:
                    nc.scalar.activation(
                        out=snew, in_=s_ps[:, par, 0:VW], func=AF.Copy
                    )
                else:
                    nc.vector.tensor_copy(out=snew, in_=s_ps[:, par, 0:VW])
                s_sb[n] = snew

            # A^T = Kp Qp^T per head (concurrent row strips), both chunks.
            for n in (n0, n1):
                sl = n % 2
                for h in range(PPC):
                    lo = h * D
                    nc.tensor.matmul(
                        opat[:, sl, h, 0:C],
                        kTp[lo : lo + D, n, :],
                        qTp[lo : lo + D, n, :],
                        start=True,
                        stop=True,
                        skip_group_check=True,
                    )
            # Masked evac of both chunks' A^T in one op.
            am = ampool.tile([C, 2, PPC, C], BF16, tag="am", name="am")
            mask_b = bass.AP(
                tensor=mask.tensor,
                offset=mask.offset,
                ap=[mask.ap[0], [0, 2], mask.ap[1], mask.ap[2]],
            )
            nc.vector.tensor_tensor(
                am, opat[:, :, :, 0:C], mask_b, op=ALU.mult
            )

            # Output accumulation (overwrites cols 0:DV of the same banks).
            for n in (n0, n1):
                sl = n % 2
                first = True
                for mm in (n - 1, n - 2):
                    if mm < 0:
                        continue
                    sprev = s_sb[mm]
                    for h in range(PPC):
                        lo = h * D
                        nc.tensor.matmul(
                            opat[:, sl, h, 0:DV],
                            qTp[lo : lo + D, n, :],
                            sprev[lo : lo + D, h * DV : (h + 1) * DV],
                            start=first,
                            stop=False,
                            skip_group_check=True,
                        )
                        first = False
                for h in range(PPC):
                    nc.tensor.matmul(
                        opat[:, sl, h, 0:DV],
                        am[:, sl, h, :],
                        vap(n, h),
                        start=first,
                        stop=(h == PPC - 1),
                        skip_group_check=True,
                    )
                    first = False

            # Evacuate both chunks' (num|z) to sbuf bf16 in one ACT op.
            nc.scalar.activation(
                out=outf[:, n0 : n0 + 2, :, :],
                in_=opat[:, :, :, 0:DV],
                func=AF.Copy,
            )

            batch = OUT_BATCHES.get(m)
            if batch is not None:
                b0, b1 = batch
                nc.sync.dma_start(out=o_d[:, b0:b1], in_=outf[:, b0:b1, :, :])

    nc.compile()
    return nc


def _get_program():
    if "nc" not in _CACHE:
        _CACHE["nc"] = _build()
    return _CACHE["nc"]


def _prep_qkv(q, k, v):
    """Per-core [C, NCH, CW] bf16 inputs: [qT | k | v+1] per chunk."""
    qr = np.asarray(q, np.float32).reshape(PAIRS, NCH, C, D)
    kr = np.asarray(k, np.float32).reshape(PAIRS, NCH, C, D)
    vr = np.asarray(v, np.float32).reshape(PAIRS, NCH, C, D)
    outs = []
    for i in range(N_CORES):
        sl = slice(i * PPC, (i + 1) * PPC)
        # qT: partition = h*64+d, free = (n, t)
        qT = qr[sl].transpose(0, 3, 1, 2).reshape(PPC * D, NCH, C)
        # k natural: partition = t, free = (n, h*64+d)
        kn = kr[sl].transpose(2, 1, 0, 3).reshape(C, NCH, PPC * D)
        # v + ones col: partition = t, free = (n, h*65+e)
        vv = np.concatenate(
            [vr[sl], np.ones((PPC, NCH, C, 1), np.float32)], axis=3
        ).transpose(2, 1, 0, 3).reshape(C, NCH, PPC * DV)
        buf = np.empty((C, NCH, CW), np.float32)
        buf[:, :, 0:QW] = qT
        buf[:, :, QW : QW + KW] = kn
        buf[:, :, QW + KW :] = vv
        outs.append(np.ascontiguousarray(buf).astype(BF))
    return outs


def run_sharded(q, k, v, trace=False, **kwargs):
    """Run on 8 cores; returns (full_output, BassKernelResults)."""
    nc = _get_program()
    ins = _prep_qkv(q, k, v)
    in_maps = [{"qkv": ins[i]} for i in range(N_CORES)]
    res = run_bass_kernel_spmd(
        nc, in_maps, core_ids=list(range(N_CORES)), trace=trace, **kwargs
    )
    # out per core: [C, NCH, PPC, DV] bf16 (num|z) -> divide -> [B,H,T,D]
    outs = []
    for i in range(N_CORES):
        oz = np.asarray(res.results[i]["out"], np.float32)
        outs.append(oz[:, :, :, 0:D] / oz[:, :, :, D:DV])
    out = np.concatenate(outs, axis=2)  # [C, NCH, PAIRS, D]
    out = out.transpose(2, 1, 0, 3).reshape(B, H, T, D)
    return np.ascontiguousarray(out, dtype=np.float32), res


def kernel(q, k, v):
    out, _ = run_sharded(q, k, v)
    return out
